# revision 1
# baseline (speedup 1.0000x reference)
"""Chamfer loss on 8 Trainium2 NeuronCores.

pred [8192,3], label [8192,3] fp32 ->
scalar = mean_i min_j ||p_i - l_j|| + mean_j min_i ||p_i - l_j||

Sharding: core k owns pred rows [k*1024:(k+1)*1024] and computes ONE
[1024 x 8192] distance block against all labels. From that single block it
extracts BOTH reductions:
  - pred-side row-mins (complete per core)   -> on-device sqrt + sum -> scalar
  - label-side column-min partials [8192]    -> output tensor; host takes the
    elementwise min across the 8 cores' partials (the "pmin" gather step) and
    finishes mean(sqrt(.)) on 8k values.

Distance tiles come straight out of an augmented K=5 matmul in fp32r
(FP22 mantissa, full PE rate at N=512):
  u_i = [-2*x, ||x||^2, 1] (stationary), v_j = [y, 1, ||y||^2] (moving)
  => (U^T V)[i,j] = ||x_i - y_j||^2 accumulated in fp32 PSUM.

Drain pipeline per row tile (PSUM reads are 1 elem/lane/cycle and allow only
one PSUM operand per instruction, so ACT does all PSUM draining while DVE
reduces from bf16 SBUF at 2-4x):
  ACT: copy psum -> bf16 SBUF tile S (4 copies of [128,2048] per row tile)
  DVE: fused tensor_scalar min-accum per copied quarter (row mins, 4x mode)
  DVE: tensor_tensor min S into two half-width column accumulators (2x mode)
Label tail: the last row tile's column accumulates land in four separate
[128,2048] quarter tiles so each dependency resolves as soon as its quarter
is drained; PE-transposes each quarter in 128x128 bf16 blocks into PSUM and
DVE min-reduces across the old partition dim -> [128,64] per-core partials.

Cost-model timeline: ~90us/core. Floor analysis: ACT psum-drain busy ~65us
(8.4M elems at 1 elem/lane/cycle @1.2GHz + per-op psum-access penalty),
ramp ~7us, DVE-serial label tail ~11us, Tile drain/barrier ~3.4us.
"""

import sys

if "/opt/trn_rl_repo" not in sys.path:
    sys.path.insert(0, "/opt/trn_rl_repo")

import numpy as np

import concourse.bacc as bacc
import concourse.mybir as mybir
from concourse import tile
from concourse.bass_utils import run_bass_kernel_spmd

F32 = mybir.dt.float32
F32R = mybir.dt.float32r
BF16 = mybir.dt.bfloat16
F16 = mybir.dt.float16
MIN = mybir.AluOpType.min
MAX = mybir.AluOpType.max
ADD = mybir.AluOpType.add
AF = mybir.ActivationFunctionType
AX_X = mybir.AxisListType.X

N_CORES = 8
N_PTS = 8192
ROWS = N_PTS // N_CORES        # pred rows owned per core
N_RTILES = ROWS // 128         # 8 row tiles of 128
PS_FREE = 2048                 # psum tile free size (4 banks)
N_HALF = N_PTS // 2            # column half handled by one accumulator
BIG = 3.0e38
DVE_TILES = ()


def _build_operands(nc, tc, const_pool, bld_pool, ps_pool, x_dram, n, ident,
                    ones_dram, scale_lhs, tag):
    """From [n,3] f32r DRAM points build augmented transposed operand tiles,
    one [5, <=4096] tile per group of 32 point-chunks, fully independent so
    the first matmuls only wait on the first group.
    lhs u = [-2x, ||x||^2, 1]; rhs v = [y, 1, ||y||^2]."""
    nt = n // 128  # point chunks of 128
    # Row pairing between lhs and rhs: row 3 = lhs ones * rhs norms,
    # row 4 = lhs norms * rhs ones. This puts the rhs transpose output
    # (fields 0-3 = coords + norms) in contiguous rows 0-3 so ONE DMA
    # assembles it (HWDGE fixed cost is ~625ns per DMA on the ramp).
    nrow = 4 if scale_lhs else 3      # norms row
    onesrow = 3 if scale_lhs else 4   # ones row
    ops = []
    for g0 in range(0, nt, 32):
        gn = min(32, nt - g0)
        op = const_pool.tile([5, gn * 128], F32R, tag=f"{tag}{g0}",
                             name=f"op_{tag}_{g0}")
        # ones row depends on nothing: issue it first so it clears the SP
        # queue before the big assembly DMA lands
        nc.sync.dma_start(
            op[onesrow : onesrow + 1, :],
            ones_dram.ap()[0:1, g0 * 128 : (g0 + gn) * 128],
        )
        stag = bld_pool.tile([128, gn, 3], F32R, tag="stag",
                             name=f"stag_{tag}_{g0}")
        # Partition-contiguous load: one 12*gn-byte descriptor per partition
        # instead of one 12-byte descriptor per point. This permutes the
        # point order (point index = p*gn + c), which is harmless: every
        # reduction downstream is order-invariant and all cores use the
        # same permutation. Pool-engine DGE queue keeps it off the SP queue.
        nc.gpsimd.dma_start(
            stag[:],
            x_dram.ap()[g0 * 128 : (g0 + gn) * 128, :]
            .rearrange("(p c) d -> p c d", p=128),
        )
        sq = bld_pool.tile([128, gn, 3], F32, tag="sq", name=f"sq_{tag}_{g0}")
        # square on DVE (idle during the ramp) to keep ACT's path short
        nc.vector.tensor_tensor(out=sq[:], in0=stag[:], in1=stag[:],
                                op=mybir.AluOpType.mult)
        # packed transpose input: partition p, free (field, chunk) contiguous
        pk = bld_pool.tile([128, 4, gn], F32R, tag="pk", name=f"pk_{tag}_{g0}")
        if scale_lhs:
            nc.vector.tensor_scalar_mul(
                pk[:, 0:3, :], stag[:].rearrange("p c d -> p d c"), -2.0
            )
        else:
            nc.vector.tensor_copy(
                pk[:, 0:3, :], stag[:].rearrange("p c d -> p d c")
            )
        with nc.allow_low_precision(reason="norms rounded to fp32r for matmul"):
            nc.vector.tensor_reduce(pk[:, 3, :], sq[:], axis=AX_X, op=ADD)
        tp = ps_pool.tile([128, 128], F32R, tag="tp")
        nc.tensor.transpose(
            tp[0 : 4 * gn, :], pk[:].rearrange("p f n -> p (f n)"), ident[:]
        )
        tpsb = bld_pool.tile([128, 128], F32R, tag="tpsb")
        nc.scalar.copy(tpsb[0 : 4 * gn, :], tp[0 : 4 * gn, :])
        if scale_lhs:
            # coords -> rows 0-2 in one DMA, norms -> row 4
            nc.sync.dma_start(
                op[0:3, :].rearrange("d (c p) -> d c p", p=128),
                tpsb[0 : 3 * gn, :],
            )
            nc.sync.dma_start(op[4:5, :], tpsb[gn * 3 : gn * 4, :])
        else:
            # coords + norms -> rows 0-3 in one DMA
            nc.sync.dma_start(
                op[0:4, :].rearrange("d (c p) -> d c p", p=128),
                tpsb[0 : 4 * gn, :],
            )
        ops.append(op)
    return ops


def build_program(repeat=1):
    nc = bacc.Bacc(
        "TRN2",
        target_bir_lowering=False,
        debug=False,
        enable_asserts=False,
        num_devices=N_CORES,
    )
    xr = nc.dram_tensor("xr", (ROWS, 3), F32R, kind="ExternalInput")
    yl = nc.dram_tensor("yl", (N_PTS, 3), F32R, kind="ExternalInput")
    ones = nc.dram_tensor("ones", (1, N_PTS), F32R, kind="ExternalInput")
    identd = nc.dram_tensor("identd", (128, 128), F32R, kind="ExternalInput")
    identbd = nc.dram_tensor("identbd", (128, 128), F16, kind="ExternalInput")
    po = nc.dram_tensor("po", (1, 1), F32, kind="ExternalOutput")
    lm = nc.dram_tensor("lm", (128, 64), F32, kind="ExternalOutput")

    with tile.TileContext(nc) as tc:
        with tc.tile_pool(name="const", bufs=1) as const_pool:
            ident = const_pool.tile([128, 128], F32R)
            nc.gpsimd.dma_start(ident[:], identd.ap())
            identb = const_pool.tile([128, 128], F16)
            nc.gpsimd.dma_start(identb[:], identbd.ap())
            ones128 = const_pool.tile([128, 1], F32)
            nc.vector.memset(ones128[:], 1.0)

            with (
                tc.tile_pool(name="bld", bufs=2) as bld_pool,
                tc.tile_pool(name="tps", bufs=2, space="PSUM") as tps_pool,
            ):
                (U,) = _build_operands(nc, tc, const_pool, bld_pool, tps_pool,
                                       xr, ROWS, ident, ones, True, "u")
                Vs = _build_operands(nc, tc, const_pool, bld_pool, tps_pool,
                                     yl, N_PTS, ident, ones, False, "v")

            with (
                tc.tile_pool(name="acc", bufs=2) as acc_pool,
                tc.tile_pool(name="s", bufs=6) as s_pool,
                tc.tile_pool(name="small", bufs=8) as small_pool,
                tc.tile_pool(name="misc", bufs=1) as misc_pool,
            ):
              for it in range(repeat):
                trash = misc_pool.tile([128, 2 * PS_FREE], F16, tag="trash",
                                       name=f"trash_{it}")
                slots_trash = misc_pool.tile([128, 4], F32, tag="slots_trash",
                                             name=f"slots_trash_{it}")
                rm_all = small_pool.tile([128, N_RTILES], F32, tag="rm_all",
                                         name=f"rm_all_{it}")
                prev_acc = [None, None]
                last_q = [None] * 4

                with tc.tile_pool(name=f"mm{it}", bufs=2,
                                  space="PSUM") as mm_pool:
                    for r in range(N_RTILES):
                        lhsT = U[:, r * 128 : (r + 1) * 128]
                        s = s_pool.tile([128, N_PTS], F16, tag="s",
                                        name=f"s_{it}_{r}")
                        slots = small_pool.tile([128, 4], F32, tag="slots",
                                                name=f"slots_{it}_{r}")
                        for b in range(4):
                            ps = mm_pool.tile([128, PS_FREE], F32, tag="mm")
                            for q in range(4):
                                c = b * 4 + q
                                nc.tensor.matmul(
                                    ps[:, q * 512 : (q + 1) * 512],
                                    lhsT,
                                    Vs[c // 8][
                                        :, (c % 8) * 512 : (c % 8 + 1) * 512
                                    ],
                                    start=True,
                                    stop=True,
                                )
                            nc.scalar.copy(
                                s[:, b * PS_FREE : (b + 1) * PS_FREE],
                                ps[:],
                            )
                            # row-min partial per quarter (4x fp16 mode):
                            # starts as soon as this quarter is copied
                            nc.vector.tensor_scalar(
                                out=trash[:, 0:PS_FREE],
                                in0=s[:, b * PS_FREE : (b + 1) * PS_FREE],
                                scalar1=BIG, scalar2=None,
                                op0=MIN, op1=MIN,
                                accum_out=slots[:, b : b + 1],
                            )
                        nc.vector.tensor_scalar(
                            out=slots_trash[:], in0=slots[:], scalar1=BIG,
                            scalar2=None, op0=MIN, op1=MIN,
                            accum_out=rm_all[:, r : r + 1],
                        )
                        # column accumulators (2x bf16 elementwise min),
                        # two independent halves; at the last row tile do
                        # half 1 first and accumulate per psum-quarter so
                        # the final updates interleave with the last ACT
                        # copies instead of serializing after them
                        for g in (0, 1):
                            sl = s[:, g * N_HALF : (g + 1) * N_HALF]
                            if r == 0:
                                acc = acc_pool.tile([128, N_HALF], F16,
                                                    tag=f"acc{g}",
                                                    name=f"acc{g}_{it}_{r}")
                                nc.vector.tensor_copy(acc[:], sl)
                                prev_acc[g] = acc
                            elif r == N_RTILES - 1:
                                # final updates land in separate quarter
                                # tiles so each transpose group's dependency
                                # resolves as soon as its quarter is done
                                for qq in range(2):
                                    qs = slice(qq * PS_FREE,
                                               (qq + 1) * PS_FREE)
                                    accq = acc_pool.tile(
                                        [128, PS_FREE], F16,
                                        tag=f"accq{g}{qq}",
                                        name=f"accq_{it}_{g}_{qq}")
                                    nc.vector.tensor_tensor(
                                        out=accq[:],
                                        in0=prev_acc[g][:, qs],
                                        in1=sl[:, qs],
                                        op=MIN,
                                    )
                                    last_q[2 * g + qq] = accq
                            else:
                                acc = acc_pool.tile([128, N_HALF], F16,
                                                    tag=f"acc{g}",
                                                    name=f"acc{g}_{it}_{r}")
                                nc.vector.tensor_tensor(
                                    out=acc[:], in0=prev_acc[g][:], in1=sl,
                                    op=MIN,
                                )
                                prev_acc[g] = acc

                    # pred tail: clamp -> sqrt -> row sum -> partition sum
                    rm_c = small_pool.tile([128, N_RTILES], F32, tag="rm_c",
                                           name=f"rm_c_{it}")
                    nc.vector.tensor_scalar_max(rm_c[:], rm_all[:], 0.0)
                    sqv = small_pool.tile([128, N_RTILES], F32, tag="sqv",
                                          name=f"sqv_{it}")
                    nc.scalar.activation(sqv[:], rm_c[:], AF.Sqrt)
                    rsum = small_pool.tile([128, 1], F32, tag="rsum",
                                           name=f"rsum_{it}")
                    nc.vector.tensor_reduce(rsum[:], sqv[:], axis=AX_X, op=ADD)
                    pps = mm_pool.tile([128, PS_FREE], F32, tag="mm",
                                       name=f"pps_{it}")
                    nc.tensor.matmul(pps[0:1, 0:1], ones128[:], rsum[:],
                                     start=True, stop=True)
                    res_sb = small_pool.tile([1, 1], F32, tag="res",
                                             name=f"res_{it}")
                    nc.scalar.copy(res_sb[:], pps[0:1, 0:1])
                    nc.sync.dma_start(po.ap()[0:1, 0:1], res_sb[:])

                # label tail: transpose acc blocks, min-reduce partitions
                lmv = misc_pool.tile([128, 64], F32, tag="lmv",
                                     name=f"lmv_{it}")
                with tc.tile_pool(name=f"tp2_{it}", bufs=4,
                                  space="PSUM") as tp2_pool:
                    for grp in (0, 1, 2, 3):  # 16 transposes per psum tile
                        tp2 = tp2_pool.tile([128, 2048], F16, tag="tp2",
                                            name=f"tp2_{it}_{grp}")
                        for t in range(16):
                            nc.tensor.transpose(
                                tp2[:, t * 128 : (t + 1) * 128],
                                last_q[grp][:, t * 128 : (t + 1) * 128],
                                identb[:],
                            )
                        nc.vector.tensor_reduce(
                            lmv[:, grp * 16 : (grp + 1) * 16],
                            tp2[:].rearrange("p (t j) -> p t j", j=128),
                            axis=AX_X,
                            op=MIN,
                        )
                nc.sync.dma_start(lm.ap(), lmv[:])

    nc.compile()
    return nc


_NC_CACHE = None


def _run(pred: np.ndarray, label: np.ndarray, trace: bool = False):
    global _NC_CACHE
    if _NC_CACHE is None:
        _NC_CACHE = build_program()
    nc = _NC_CACHE

    pred = np.ascontiguousarray(pred, dtype=np.float32)
    label = np.ascontiguousarray(label, dtype=np.float32)
    ones = np.ones((1, N_PTS), np.float32)
    ident = np.eye(128, dtype=np.float32)
    import ml_dtypes
    identb = np.eye(128, dtype=np.float16)

    in_maps = []
    for k in range(N_CORES):
        sl = slice(k * ROWS, (k + 1) * ROWS)
        in_maps.append(
            {"xr": pred[sl], "yl": label, "ones": ones, "identd": ident,
             "identbd": identb}
        )

    # The axon-tunneled device occasionally reports a transient
    # NRT_EXEC_UNIT_UNRECOVERABLE on the first touch after idling; a retry
    # on a fresh dispatch succeeds.
    last_err = None
    for attempt in range(3):
        try:
            res = run_bass_kernel_spmd(
                nc, in_maps, core_ids=list(range(N_CORES)), trace=trace
            )
            break
        except Exception as e:  # noqa: BLE001
            last_err = e
            import time as _time

            _time.sleep(2.0 * (attempt + 1))
    else:
        raise last_err
    po = np.stack([res.results[k]["po"][0, 0] for k in range(N_CORES)])
    lmp = np.stack([res.results[k]["lm"] for k in range(N_CORES)])

    pred_side = float(po.sum(dtype=np.float64)) / N_PTS
    lab_d2 = np.minimum.reduce(lmp.astype(np.float64), axis=0)  # [128, 64]
    lab_side = float(np.sqrt(np.clip(lab_d2, 0.0, None)).sum()) / N_PTS
    return np.float32(pred_side + lab_side), res


def kernel(pred: np.ndarray, label: np.ndarray) -> np.ndarray:
    return _run(pred, label)[0]



# revision 4
# speedup vs baseline: 1.1998x; 1.1998x over previous
"""Chamfer loss on 8 Trainium2 NeuronCores (v2).

pred [8192,3], label [8192,3] fp32 ->
scalar = mean_i min_j ||p_i - l_j|| + mean_j min_i ||p_i - l_j||

Sharding: core k owns pred rows [k*1024:(k+1)*1024] and computes one
[1024 x 8192] squared-distance block against all labels via an augmented
K=5 fp32r matmul (host-precomputed operands):
  U[5,1024] = [-2x | 1 | ||x||^2]   (stationary, 128-row tiles)
  V[5,8192] = [ y | ||y||^2 | 1 ]   (moving)
so (U^T V)[i,j] = ||x_i - y_j||^2 accumulated in fp32 PSUM.

Column-group loop (outer) over widths [2048,2048,2048,1536,512]; row-tile
loop (inner, 8 tiles of 128 rows). Per cell [128,W]:
  - matmuls (512-wide) into a [128,2048] PSUM tile (bufs=2)
  - drain to SBUF f16: ACT copy (+separate DVE 4x row-min scan) or a DVE
    tensor_scalar straight from PSUM (fused drain+clamp+row-min accum)
  - column-min fold into the group accumulator: DVE (2x f16) or Pool
Group tail: Pool tensor_reduce over the PARTITION axis (AxisListType.C)
gives per-label partial mins [1,W] with no transpose; the last (512-wide)
group keeps the terminal chain short. Pred tail: fold row-min slots,
clamp, sqrt (ACT), row-sum, PE ones-matmul partition sum -> scalar.

Host combines: po sums across cores; label partials pmin across cores,
then sqrt+mean. All engine assignments live in the tables below; costs
are balanced against the TimelineSim instruction cost model
(ACT 0.83ns/elem drain, DVE 4x scan / 2x fold, Pool 1.39ns/elem).
"""

import sys

if "/opt/trn_rl_repo" not in sys.path:
    sys.path.insert(0, "/opt/trn_rl_repo")

import numpy as np

import concourse.bacc as bacc
import concourse.mybir as mybir
from concourse import tile
from concourse.bass_utils import run_bass_kernel_spmd

F32 = mybir.dt.float32
F32R = mybir.dt.float32r
F16 = mybir.dt.float16
MIN = mybir.AluOpType.min
MAX = mybir.AluOpType.max
ADD = mybir.AluOpType.add
AF = mybir.ActivationFunctionType
AX_X = mybir.AxisListType.X
AX_C = mybir.AxisListType.C

N_CORES = 8
N_PTS = 8192
ROWS = N_PTS // N_CORES          # 1024 pred rows per core
N_RT = ROWS // 128               # 8 row tiles

# column groups: (start, width)
GROUPS = [(0, 2048), (2048, 2048), (4096, 2048), (6144, 1536), (7680, 512)]

# drain engine per (group, row tile): 'A' = ACT copy + DVE scan,
# 'V' = DVE tensor_scalar fused drain+scan from PSUM
DRAIN = [
    "AAVAAAAA",   # g0
    "AAAAAVAA",   # g1
    "AAVAAAAA",   # g2
    "AAAAAAAA",   # g3 (1536)
    "AAAAAAAA",   # g4 (512, terminal: keep ACT, cheap cells)
]
# fold engine per (group, row tile>=1): 'V' = DVE, 'P' = Pool
# Pool ucode rejects min/max TensorTensor/TensorScalar; folds are DVE-only.
FOLD = [
    "-VVVVVVV",
    "-VVVVVVV",
    "-VVVVVVV",
    "-VVVVVVV",
    "-VVVVVVV",
]


def build_program():
    nc = bacc.Bacc(
        "TRN2",
        target_bir_lowering=False,
        debug=False,
        enable_asserts=False,
        num_devices=N_CORES,
    )
    u_d = nc.dram_tensor("u_d", (5, ROWS), F32R, kind="ExternalInput")
    v_d = nc.dram_tensor("v_d", (5, N_PTS), F32R, kind="ExternalInput")
    po = nc.dram_tensor("po", (1, 1), F32, kind="ExternalOutput")
    lm = nc.dram_tensor("lm", (1, N_PTS), F32, kind="ExternalOutput")

    with tile.TileContext(nc) as tc:
        with tc.tile_pool(name="const", bufs=1) as const_pool:
            U = const_pool.tile([5, ROWS], F32R)
            nc.sync.dma_start(U[:], u_d.ap())
            Vs = []
            for g, (c0, w) in enumerate(GROUPS):
                V = const_pool.tile([5, w], F32R, tag=f"v{g}", name=f"v_{g}")
                nc.sync.dma_start(V[:], v_d.ap()[:, c0 : c0 + w])
                Vs.append(V)
            ones128 = const_pool.tile([128, 1], F32)
            nc.vector.memset(ones128[:], 1.0)
            # preload the Sqrt activation table off the critical path
            sqwarm = const_pool.tile([128, 1], F32)
            nc.scalar.activation(sqwarm[:], ones128[:], AF.Sqrt)

            with (
                tc.tile_pool(name="mm", bufs=2, space="PSUM") as mm_pool,
                tc.tile_pool(name="s", bufs=3) as s_pool,
                tc.tile_pool(name="acc", bufs=2) as acc_pool,
                tc.tile_pool(name="small", bufs=2) as small_pool,
                tc.tile_pool(name="misc", bufs=1) as misc_pool,
            ):
                trash = misc_pool.tile([128, 2048], F16)
                lmv = misc_pool.tile([1, N_PTS], F32)
                slots = misc_pool.tile([128, N_RT * len(GROUPS)], F32)

                for g, (c0, w) in enumerate(GROUPS):
                    acc = None
                    for r in range(N_RT):
                        mm = mm_pool.tile([128, 2048], F32, tag="mm")
                        for j in range(w // 512):
                            nc.tensor.matmul(
                                mm[:, j * 512 : (j + 1) * 512],
                                U[:, r * 128 : (r + 1) * 128],
                                Vs[g][:, j * 512 : (j + 1) * 512],
                                start=True,
                                stop=True,
                            )
                        if r == 0:
                            dst = acc_pool.tile([128, w], F16, tag=f"acc{g}",
                                                name=f"acc_{g}_{r}")
                        else:
                            dst = s_pool.tile([128, w], F16, tag=f"s{g}",
                                              name=f"s_{g}_{r}")
                        slot = slots[:, g * N_RT + r : g * N_RT + r + 1]
                        if DRAIN[g][r] == "A":
                            nc.scalar.copy(dst[:], mm[:, :w])
                            nc.vector.tensor_scalar(
                                out=trash[:, :w], in0=dst[:],
                                scalar1=0.0, scalar2=None,
                                op0=MIN, op1=MAX, accum_out=slot,
                            )
                        else:
                            nc.vector.tensor_scalar(
                                out=dst[:], in0=mm[:, :w],
                                scalar1=0.0, scalar2=None,
                                op0=MIN, op1=MAX, accum_out=slot,
                            )
                        if r == 0:
                            acc = dst
                        else:
                            nacc = acc_pool.tile([128, w], F16, tag=f"acc{g}",
                                                 name=f"acc_{g}_{r}")
                            eng = nc.vector if FOLD[g][r] == "V" else nc.gpsimd
                            eng.tensor_tensor(out=nacc[:], in0=acc[:],
                                              in1=dst[:], op=MAX)
                            acc = nacc
                    # group tail: Pool partition-axis reduce -> [1, w]
                    nc.gpsimd.tensor_reduce(
                        lmv[0:1, c0 : c0 + w], acc[:], axis=AX_C, op=MAX
                    )
                    nc.sync.dma_start(
                        lm.ap()[0:1, c0 : c0 + w], lmv[0:1, c0 : c0 + w]
                    )

                # pred tail: fold slots (5 groups * 8 rt) -> [128, 8] row mins
                ns = len(GROUPS)
                rm = small_pool.tile([128, N_RT], F32, tag="rm")
                nc.vector.tensor_tensor(
                    out=rm[:],
                    in0=slots[:, 0:N_RT],
                    in1=slots[:, N_RT : 2 * N_RT],
                    op=MAX,
                )
                for g in range(2, ns):
                    nrm = small_pool.tile([128, N_RT], F32, tag="rm",
                                          name=f"rm_{g}")
                    nc.vector.tensor_tensor(
                        out=nrm[:], in0=rm[:],
                        in1=slots[:, g * N_RT : (g + 1) * N_RT], op=MAX,
                    )
                    rm = nrm
                sq = small_pool.tile([128, N_RT], F32, tag="sq")
                nc.scalar.activation(sq[:], rm[:], AF.Sqrt, scale=-1.0)
                rsum = small_pool.tile([128, 1], F32, tag="rsum")
                nc.vector.tensor_reduce(rsum[:], sq[:], axis=AX_X, op=ADD)
                res_sb = small_pool.tile([1, 1], F32, tag="res")
                nc.gpsimd.tensor_reduce(res_sb[:], rsum[:], axis=AX_C, op=ADD)
                nc.scalar.dma_start(po.ap()[0:1, 0:1], res_sb[:])

    nc.compile()
    return nc


_NC_CACHE = None


def _run(pred: np.ndarray, label: np.ndarray, trace: bool = False):
    global _NC_CACHE
    if _NC_CACHE is None:
        _NC_CACHE = build_program()
    nc = _NC_CACHE

    pred = np.ascontiguousarray(pred, dtype=np.float32)
    label = np.ascontiguousarray(label, dtype=np.float32)

    # host-side augmented operands
    y2 = (label * label).sum(axis=1)
    V = np.empty((5, N_PTS), np.float32)
    V[0:3] = label.T
    V[3] = y2
    V[4] = 1.0

    in_maps = []
    for k in range(N_CORES):
        x = pred[k * ROWS : (k + 1) * ROWS]
        x2 = (x * x).sum(axis=1)
        Uk = np.empty((5, ROWS), np.float32)
        Uk[0:3] = 2.0 * x.T
        Uk[3] = -1.0
        Uk[4] = -x2
        in_maps.append({"u_d": Uk, "v_d": V})

    # transient NRT_EXEC_UNIT_UNRECOVERABLE on first touch after idling;
    # retry on a fresh dispatch succeeds.
    last_err = None
    for attempt in range(3):
        try:
            res = run_bass_kernel_spmd(
                nc, in_maps, core_ids=list(range(N_CORES)), trace=trace
            )
            break
        except Exception as e:  # noqa: BLE001
            last_err = e
            import time as _time

            _time.sleep(2.0 * (attempt + 1))
    else:
        raise last_err

    po = np.stack([res.results[k]["po"][0, 0] for k in range(N_CORES)])
    lmp = np.stack([res.results[k]["lm"][0] for k in range(N_CORES)])

    pred_side = float(po.sum(dtype=np.float64)) / N_PTS
    lab_d2 = -np.maximum.reduce(lmp.astype(np.float64), axis=0)  # [8192]
    lab_side = float(np.sqrt(np.clip(lab_d2, 0.0, None)).sum()) / N_PTS
    return np.float32(pred_side + lab_side), res


def kernel(pred: np.ndarray, label: np.ndarray) -> np.ndarray:
    return _run(pred, label)[0]
